# revision 21
# baseline (speedup 1.0000x reference)
"""Trainium2 Bass kernel for the dual-modality dense transformer block.

Problem (hardcoded shapes): B=8, L=1024, H=512, NH=8, HD=64.
Sharding: pure data-parallel over batch B=8 across the 8 NeuronCores.

Device algorithm (per core, one batch element):
  - Feature-major activations ([H, L]) throughout; host pre-transposes
    inputs and weights. q-side weights pre-scaled by 1/(sqrt(HD)*16) so
    score psums arrive as s/16: the ACT exp tiles use scale=16, the DVE
    exp tiles use a custom one-instruction polynomial op
    ((c2 y^2 + c1 y + c0)^16 ~= exp(16 y)) registered at import time.
  - Softmax exp is split between the Scalar (ACT) and Vector (DVE)
    engines per key-tile to balance the two engines; exp output is
    fp8e4m3.
  - PV matmuls run in fp8 DoubleRow mode (two 128-key planes per
    matmul), with a ones column (M=65) producing softmax denominators
    on psum partition 64 for free.
  - Normalization: fast-reciprocal directly from PSUM, partition
    broadcast via a DRAM scratch row, applied during the fp32 PSUM
    eviction on DVE (lag-1 so the PE never stalls on it). The 0.5
    pair-averaging and all V-side biases are folded into downstream
    weights/biases host-side.
"""

import numpy as np
import ml_dtypes

import concourse.bass as bass
import concourse.tile as tile
from concourse import bacc, mybir
from concourse.bass_utils import run_bass_kernel_spmd

B, L, H, NH, HD = 8, 1024, 512, 8, 64
BF = mybir.dt.bfloat16
F32 = mybir.dt.float32
F8 = mybir.dt.float8e4
Exp = mybir.ActivationFunctionType.Exp
DR = mybir.MatmulPerfMode.DoubleRow
bf16 = ml_dtypes.bfloat16

N_CORES = 8

# ---- custom DVE exp op: ((c2 y^2 + c1 y + c0))^16 ~= exp(16 y) ------------
EXP_C = dict(s0=0.49805024, s1=1.0077711, imm2=1.000121)


def _register_exp_op():
    import concourse.dve_ops as dve_ops
    from concourse.dve_spec import Spec, Src0, C0, C1, C2, sq, lower, _has_src1
    from concourse.dve_uop import DveOpSpec

    name = "EXP16Q_ANT"
    for op in dve_ops.OPS:
        if op.name == name:
            return op
    p = (Src0 * C0 + C1) * Src0 + C2
    body = sq(sq(sq(sq(p))))

    def _ref(in0, in1, s0, s1, imm2):
        pp = ((in0 * s0 + s1) * in0 + imm2).astype(np.float32)
        for _ in range(4):
            pp = (pp * pp).astype(np.float32)
        return pp

    spec = Spec(body=body, reference=_ref)
    row = 1 + len(dve_ops.OPS)
    assert row < 0x20
    shas = {}
    for ver in ("v3", "v4"):
        try:
            tmp = DveOpSpec(name=name, opcode=row, uops=lower(spec, ver=ver),
                            rd1_en=_has_src1(spec))
            shas[ver] = tmp.sha(ver)
        except Exception:
            pass
    op = dve_ops.DveOp(name, spec, subdim=False, uops_sha=shas)
    dve_ops.OPS.append(op)
    dve_ops.CUSTOM_DVE_SPECS[name] = spec
    dve_ops._SUB_OPCODE_FOR_NAME[name] = row
    return op


EXP_OP = _register_exp_op()

# key-tiles handled by the DVE exp op (rest go to ACT)
DVE_JTS = (2, 5)
DEBUG_DUMPS = False


def _emit(tc, d):
    nc = tc.nc
    import contextlib

    ctx = contextlib.ExitStack()
    with ctx:
        const = ctx.enter_context(tc.tile_pool(name="const", bufs=1))
        acts = ctx.enter_context(tc.tile_pool(name="acts", bufs=1))
        spool = ctx.enter_context(tc.tile_pool(name="spool", bufs=2))
        opool = ctx.enter_context(tc.tile_pool(name="opool", bufs=1))
        expool = ctx.enter_context(tc.tile_pool(name="expool", bufs=2))
        small = ctx.enter_context(tc.tile_pool(name="small", bufs=2))
        dscr = ctx.enter_context(tc.tile_pool(name="dscr", bufs=8, space="DRAM"))
        pmm = ctx.enter_context(tc.tile_pool(name="pmm", bufs=2, space="PSUM"))
        pctx = ctx.enter_context(tc.tile_pool(name="pctx", bufs=2, space="PSUM"))

        # ---- constants / inputs into SBUF ----
        def load(name, p_chunks, free, dt=BF):
            t = const.tile([128, p_chunks, free], dt, tag=name)
            src_r = d[name].rearrange("(c p) n -> p c n", p=128)
            for c in range(p_chunks):
                nc.sync.dma_start(out=t[:, c, :], in_=src_r[:, c, :])
            return t

        def load_act(name, p_chunks, free, tag):
            t = acts.tile([128, p_chunks, free], BF, tag=tag)
            src_r = d[name].rearrange("(c p) n -> p c n", p=128)
            for c in range(p_chunks):
                nc.sync.dma_start(out=t[:, c, :], in_=src_r[:, c, :])
            return t

        def load2d(name, p, free, dt):
            t = const.tile([p, free], dt, tag=name)
            nc.sync.dma_start(out=t, in_=d[name])
            return t

        xt = load_act("xT", 4, L, "xT")
        w_qim = load("w_qim", 4, H)
        b_qim = load2d("b_qim", 128, 4, F32)
        w_kim = load("w_kim", 4, H)
        b_kim = load2d("b_kim", 128, 4, F32)
        w_vim = load("w_vim", 4, H)
        tt = load_act("tT", 4, L, "tT")
        w_qtx = load("w_qtx", 4, H)
        b_qtx = load2d("b_qtx", 128, 4, F32)
        w_ktx = load("w_ktx", 4, H)
        b_ktx = load2d("b_ktx", 128, 4, F32)
        w_vtx = load("w_vtx", 4, H)
        w_oim = load("w_oim", 4, H)
        b_oim = load2d("b_oim", 128, 4, F32)
        w_otx = load("w_otx", 4, H)
        b_otx = load2d("b_otx", 128, 4, F32)
        w_cat = load("w_cat", 8, H)
        b_cat = load2d("b_cat", 128, 4, F32)
        w_ip = load("w_ip", 4, 3 * H)
        b_ipqk = load2d("b_ipqk", 128, 8, F32)
        w_op = load("w_op", 4, H)
        r_op = load2d("r_op", 1, H, BF)

        ones_row = const.tile([1, 128], BF, tag="ones_row")
        nc.vector.memset(ones_row, 1.0)

        # ---- helpers ----
        def proj_T(dst, dst_off, src, nk, w, w_off, bias, bias_off, single=False):
            """feature-major linear: dst[:, dst_off+m, :] = (w.T @ src) + bias.
            k-outer so consecutive matmul pairs share the stationary operand.
            single=True: dst_off is one m-block index (filler granularity)."""
            ms = [dst_off] if single else range(4)
            off = 0 if single else dst_off
            for m in ms:
                ps = pmm.tile([128, 1024], F32, tag="mm")
                for k in range(nk):
                    for n in range(2):
                        nc.tensor.matmul(
                            ps[:, n * 512 : (n + 1) * 512],
                            w[:, k, w_off + m * 128 : w_off + (m + 1) * 128],
                            src[:, k, n * 512 : (n + 1) * 512],
                            start=(k == 0),
                            stop=(k == nk - 1),
                            skip_group_check=True,
                        )
                nc.vector.tensor_scalar_add(
                    dst[:, off + m, :], ps, bias[:, bias_off + m : bias_off + m + 1]
                )

        def proj_N(dst, lc2, src, w, w_off):
            """natural-orientation linear into the fp8 ones-augmented V layout
            for one token-block pair lc2: dst [128, 8(jt), 8(head), 80]."""
            ps = pmm.tile([128, 1024], F32, tag="mm")
            for h in range(2):
                lc = lc2 * 2 + h
                for k in range(4):
                    nc.tensor.matmul(
                        ps[:, h * 512 : (h + 1) * 512],
                        src[:, k, lc * 128 : (lc + 1) * 128],
                        w[:, k, w_off : w_off + 512],
                        start=(k == 0),
                        stop=(k == 3),
                        skip_group_check=True,
                    )
            nc.vector.tensor_copy(
                out=dst[:, lc2 * 2 : lc2 * 2 + 2, :, 0:64],
                in_=ps.rearrange("p (a b) -> p a b", a=2),
            )

        # Normalization of a group is deferred one stage (lag-1) so the PE
        # never stalls on the DVE reciprocal chain.
        pending = [None]
        gidx = [0]

        def flush():
            if pending[0] is not None:
                pending[0]()
                pending[0] = None

        def attention(qT, kT, vN, s_src, s_dst, fillers, mid_hook=None):
            """One multi-head attention. If s_src is None: s_dst = ctx'.
            Else: s_dst = s_src + ctx' (non-aliased accumulate).

            vN is fp8 ones-augmented [128, 8(jt), 8(head), 80]; PV runs in
            DoubleRow mode over key-tile pairs; the ones column (M=65)
            produces softmax denominators on psum partition 64."""
            prev_pv = [None]

            def group(ih, p):
                gidx[0] += 1
                dve_jts = DVE_JTS if gidx[0] % 2 else DVE_JTS + (7,)
                i0 = ih * 512
                ex = expool.tile([128, 8, 1024], F8, tag="exp")
                for jt in range(8):
                    ps = pmm.tile([128, 1024], F32, tag="mm")
                    for hh in range(2):
                        nc.tensor.matmul(
                            ps[:, hh * 512 : (hh + 1) * 512],
                            kT[hh * 64 : (hh + 1) * 64, p, jt * 128 : (jt + 1) * 128],
                            qT[hh * 64 : (hh + 1) * 64, p, i0 : i0 + 512],
                            start=True,
                            stop=True,
                            tile_position=(hh * 64, 0),
                        )
                    if jt in dve_jts:
                        nc.vector._custom_dve(EXP_OP, out=ex[:, jt, :], in0=ps, **EXP_C)
                    else:
                        nc.scalar.activation(ex[:, jt, :], ps, Exp, scale=16.0)

                def emit_pv():
                    cps = pctx.tile([128, 1024], F32, tag="ctx")
                    for jtp in range(4):
                        for hh in range(2):
                            nc.tensor.matmul(
                                cps[0:65, hh * 512 : (hh + 1) * 512],
                                vN[:, 2 * jtp : 2 * jtp + 2, p * 2 + hh, 0:65],
                                ex[:, 2 * jtp : 2 * jtp + 2, hh * 512 : (hh + 1) * 512],
                                start=(jtp == 0),
                                stop=(jtp == 3),
                                perf_mode=DR,
                                skip_group_check=True,
                            )
                    flush()

                    def normalize(cps=cps, p=p, i0=i0):
                        # custom-DVE ops require base partition 0, so launder
                        # the den row (psum partition 64) to SBUF first; then
                        # bounce it through DRAM reshaped to [64, 16] so the
                        # reciprocal runs on 64 lanes instead of 1
                        den = small.tile([1, 1024], F32, tag="den")
                        nc.vector.tensor_copy(out=den, in_=cps[64:65, :])
                        dr0 = dscr.tile([1, 1024], F32, tag="dr0")
                        nc.sync.dma_start(out=dr0, in_=den)
                        den_rs = small.tile([64, 16], F32, tag="denrs")
                        nc.sync.dma_start(
                            out=den_rs, in_=dr0.rearrange("o (p n) -> (o p) n", p=64))
                        rcs = small.tile([64, 16], F32, tag="rcs")
                        nc.vector.reciprocal_approx_fast(out=rcs, in_=den_rs)
                        dr = dscr.tile([1, 1024], F32, tag="dr")
                        nc.sync.dma_start(
                            out=dr.rearrange("o (p n) -> (o p) n", p=64), in_=rcs)
                        bcs = small.tile([128, 512], F32, tag="bcs")
                        for hh in range(2):
                            sl = dr[0:1, hh * 512 : (hh + 1) * 512]
                            bsrc = bass.AP(
                                tensor=sl.tensor, offset=sl.offset,
                                ap=[[0, 64]] + [list(a) for a in sl.ap[1:]],
                            )
                            nc.sync.dma_start(out=bcs[hh * 64 : (hh + 1) * 64, :], in_=bsrc)
                        o = s_dst[:, p, i0 : i0 + 512]
                        if s_src is None:
                            nc.vector.tensor_mul(o[0:64, :], cps[0:64, 0:512], bcs[0:64, :])
                            nc.vector.tensor_mul(o[64:128, :], cps[0:64, 512:1024], bcs[64:128, :])
                        else:
                            tmp = small.tile([128, 512], BF, tag="tmp")
                            nc.vector.tensor_mul(tmp[0:64, :], cps[0:64, 0:512], bcs[0:64, :])
                            nc.vector.tensor_mul(tmp[64:128, :], cps[0:64, 512:1024], bcs[64:128, :])
                            nc.vector.tensor_add(o, s_src[:, p, i0 : i0 + 512], tmp)

                    pending[0] = normalize

                return emit_pv

            slots = 8
            for ih in range(2):
                for p in range(4):
                    pv = group(ih, p)
                    if prev_pv[0] is not None:
                        prev_pv[0]()
                    prev_pv[0] = pv
                    if ih == 1 and p == 2 and mid_hook is not None:
                        mid_hook()
                    n_pop = -(-len(fillers) // slots)  # ceil: spread evenly
                    for _ in range(n_pop):
                        fillers.pop(0)()
                    slots -= 1
            prev_pv[0]()
            flush()  # all normalizes emitted before anything reads s_dst
            while fillers:
                fillers.pop(0)()

        # ---- the network ----
        q_im = acts.tile([128, 4, L], BF, tag="q_im")
        k_im = acts.tile([128, 4, L], BF, tag="k_im")
        v_im = acts.tile([128, 8, 8, 80], F8, tag="v_im")
        nc.vector.memset(v_im[:, :, :, 64:65], 1.0)
        q_tx = acts.tile([128, 4, L], BF, tag="q_tx")
        k_tx = acts.tile([128, 4, L], BF, tag="k_tx")
        v_tx = acts.tile([128, 8, 8, 80], F8, tag="v_tx")
        nc.vector.memset(v_tx[:, :, :, 64:65], 1.0)

        for lc2 in range(4):
            proj_N(v_im, lc2, xt, w_vim, 0)
        proj_T(q_im, 0, xt, 4, w_qim, 0, b_qim, 0, single=True)
        proj_T(k_im, 0, xt, 4, w_kim, 0, b_kim, 0, single=True)

        s_img = spool.tile([128, 4, L], BF, tag="s")
        s_img2 = spool.tile([128, 4, L], BF, tag="s2")

        # fillers for A_img: rest of q_im/k_im (1-group lead), then text stream
        fill = []
        for m in range(1, 4):
            fill.append(lambda m=m: proj_T(q_im, m, xt, 4, w_qim, 0, b_qim, 0, single=True))
            fill.append(lambda m=m: proj_T(k_im, m, xt, 4, w_kim, 0, b_kim, 0, single=True))
        for m in range(4):
            fill.append(lambda m=m: proj_T(q_tx, m, tt, 4, w_qtx, 0, b_qtx, 0, single=True))
        for m in range(4):
            fill.append(lambda m=m: proj_T(k_tx, m, tt, 4, w_ktx, 0, b_ktx, 0, single=True))
        for lc2 in range(4):
            fill.append(lambda lc2=lc2: proj_N(v_tx, lc2, tt, w_vtx, 0))

        attention(q_im, k_im, v_im, None, s_img, fill)       # ctx_img
        attention(q_im, k_tx, v_tx, s_img, s_img2, [])       # + ctx_it

        if DEBUG_DUMPS:
            nc.sync.dma_start(out=d["dbg_qim"], in_=q_im.rearrange("p a b -> p (a b)"))
            nc.sync.dma_start(out=d["dbg_simg"], in_=s_img.rearrange("p a b -> p (a b)"))
            nc.sync.dma_start(out=d["dbg_vim"], in_=v_im.rearrange("p a b c -> p (a b c)"))

        s_txt = spool.tile([128, 4, L], BF, tag="s")
        s_txt2 = spool.tile([128, 4, L], BF, tag="s2")
        cat_a = acts.tile([128, 4, L], BF, tag="xT")

        fill = [lambda m=m: proj_T(cat_a, m, s_img2, 4, w_oim, 0, b_oim, 0, single=True)
                for m in range(4)]
        attention(q_tx, k_tx, v_tx, None, s_txt, fill)       # ctx_txt
        attention(q_tx, k_im, v_im, s_txt, s_txt2, [])       # + ctx_ti

        cat_b = acts.tile([128, 4, L], BF, tag="tT")
        proj_T(cat_b, 0, s_txt2, 4, w_otx, 0, b_otx, 0)

        out_t = opool.tile([128, 4, L], BF, tag="out")
        for m in range(4):
            ps = pmm.tile([128, 1024], F32, tag="mm")
            for k in range(8):
                srck = cat_a if k < 4 else cat_b
                for n in range(2):
                    nc.tensor.matmul(
                        ps[:, n * 512 : (n + 1) * 512],
                        w_cat[:, k, m * 128 : (m + 1) * 128],
                        srck[:, k % 4, n * 512 : (n + 1) * 512],
                        start=(k == 0),
                        stop=(k == 7),
                        skip_group_check=True,
                    )
            nc.vector.tensor_scalar_add(out_t[:, m, :], ps, b_cat[:, m : m + 1])

        if DEBUG_DUMPS:
            nc.sync.dma_start(out=d["dbg_cata"], in_=cat_a.rearrange("p a b -> p (a b)"))
            nc.sync.dma_start(out=d["dbg_catb"], in_=cat_b.rearrange("p a b -> p (a b)"))
            nc.sync.dma_start(out=d["dbg_simg2"], in_=s_img2.rearrange("p a b -> p (a b)"))
            nc.sync.dma_start(out=d["dbg_stxt2"], in_=s_txt2.rearrange("p a b -> p (a b)"))
            nc.sync.dma_start(out=d["dbg_outt"], in_=out_t.rearrange("p a b -> p (a b)"))

        q_pl = acts.tile([128, 4, L], BF, tag="q_im")
        k_pl = acts.tile([128, 4, L], BF, tag="q_tx")
        v_pl = acts.tile([128, 8, 8, 80], F8, tag="v_im")
        nc.vector.memset(v_pl[:, :, :, 64:65], 1.0)
        proj_T(q_pl, 0, out_t, 4, w_ip, 0, b_ipqk, 0)
        proj_T(k_pl, 0, out_t, 4, w_ip, 512, b_ipqk, 4)
        for lc2 in range(4):
            proj_N(v_pl, lc2, out_t, w_ip, 1024)

        ctx_p = spool.tile([128, 4, L], BF, tag="s")

        def emit_out_proj(lcs):
            for lc in lcs:
                ps = pmm.tile([128, 1024], F32, tag="mm")
                for k in range(4):
                    nc.tensor.matmul(
                        ps[:, 0:512],
                        ctx_p[:, k, lc * 128 : (lc + 1) * 128],
                        w_op[:, k, :],
                        start=(k == 0),
                        stop=False,
                        skip_group_check=True,
                    )
                nc.tensor.matmul(
                    ps[:, 0:512], ones_row, r_op, start=False, stop=True,
                    skip_group_check=True,
                )
                res = small.tile([128, 512], F32, tag="res")
                nc.vector.tensor_copy(out=res, in_=ps[:, 0:512])
                nc.sync.dma_start(out=d["out"][lc * 128 : (lc + 1) * 128, :], in_=res)

        def pool_mid():
            emit_out_proj(range(4))

        attention(q_pl, k_pl, v_pl, None, ctx_p, [], mid_hook=pool_mid)
        emit_out_proj(range(4, 8))


_PROGRAM = None


def _build_program():
    global _PROGRAM
    if _PROGRAM is not None:
        return _PROGRAM
    nc = bacc.Bacc("TRN2", target_bir_lowering=False, debug=False)
    d = {}

    def din(name, shape, dt):
        d[name] = nc.dram_tensor(name, list(shape), dt, kind="ExternalInput").ap()

    din("xT", (H, L), BF)
    din("tT", (H, L), BF)
    for n in ("w_qim", "w_kim", "w_vim", "w_qtx", "w_ktx", "w_vtx", "w_oim", "w_otx"):
        din(n, (H, H), BF)
    din("w_cat", (2 * H, H), BF)
    din("w_ip", (H, 3 * H), BF)
    din("w_op", (H, H), BF)
    for n in ("b_qim", "b_kim", "b_qtx", "b_ktx", "b_oim", "b_otx", "b_cat"):
        din(n, (128, 4), F32)
    din("b_ipqk", (128, 8), F32)
    din("r_op", (1, H), BF)
    d["out"] = nc.dram_tensor("out", [L, H], F32, kind="ExternalOutput").ap()
    if DEBUG_DUMPS:
        d["dbg_qim"] = nc.dram_tensor("dbg_qim", [128, 4 * L], BF, kind="ExternalOutput").ap()
        d["dbg_simg"] = nc.dram_tensor("dbg_simg", [128, 4 * L], BF, kind="ExternalOutput").ap()
        d["dbg_vim"] = nc.dram_tensor("dbg_vim", [128, 8 * 8 * 80], F8, kind="ExternalOutput").ap()
        d["dbg_rc"] = nc.dram_tensor("dbg_rc", [1, 1024], F32, kind="ExternalOutput").ap()
        for n in ("dbg_simg2", "dbg_stxt2", "dbg_outt", "dbg_cata", "dbg_catb"):
            d[n] = nc.dram_tensor(n, [128, 4 * L], BF, kind="ExternalOutput").ap()

    with tile.TileContext(nc) as tc:
        _emit(tc, d)
    nc.compile()
    _PROGRAM = nc
    return nc


def _host_prep(inputs):
    f = lambda x: np.asarray(x, np.float32)

    def wT(w, scale=None):
        w = f(w)
        if scale is not None:
            w = w * scale
        return np.ascontiguousarray(w.T).astype(bf16)

    def bcol(b, scale=None):
        b = f(b)
        if scale is not None:
            b = b * scale
        return np.ascontiguousarray(b.reshape(-1, 128).T.astype(np.float32))

    s = 1.0 / (np.sqrt(HD) * 16.0)
    ipw = f(inputs["in_proj_w"]).copy()
    ipw[0:H] *= s
    ipb = f(inputs["in_proj_b"]).copy()
    ipb[0:H] *= s

    # fold V biases + the 0.5 pair-averaging into out_img/out_txt
    bv_avg = (f(inputs["b_v_img"]) + f(inputs["b_v_txt"])) * 0.5
    w_oim_f = f(inputs["w_out_img"])
    w_otx_f = f(inputs["w_out_txt"])
    b_oim_f = f(inputs["b_out_img"]) + w_oim_f @ bv_avg
    b_otx_f = f(inputs["b_out_txt"]) + w_otx_f @ bv_avg
    # fold pooling V bias into out_proj bias
    b_op_f = f(inputs["out_proj_b"]) + f(inputs["out_proj_w"]) @ ipb[2 * H : 3 * H]

    shared = {
        "w_qim": wT(inputs["w_q_img"], s),
        "w_kim": wT(inputs["w_k_img"]),
        "w_vim": wT(inputs["w_v_img"]),
        "w_qtx": wT(inputs["w_q_txt"], s),
        "w_ktx": wT(inputs["w_k_txt"]),
        "w_vtx": wT(inputs["w_v_txt"]),
        "w_oim": wT(w_oim_f, 0.5),
        "w_otx": wT(w_otx_f, 0.5),
        "w_cat": wT(inputs["w_cat"]),
        "w_ip": wT(ipw),
        "w_op": wT(inputs["out_proj_w"]),
        "b_qim": bcol(inputs["b_q_img"], s),
        "b_kim": bcol(inputs["b_k_img"]),
        "b_qtx": bcol(inputs["b_q_txt"], s),
        "b_ktx": bcol(inputs["b_k_txt"]),
        "b_oim": bcol(b_oim_f),
        "b_otx": bcol(b_otx_f),
        "b_cat": bcol(inputs["b_cat"]),
        "b_ipqk": bcol(ipb[0 : 2 * H]),
        "r_op": b_op_f.astype(bf16).reshape(1, -1),
    }
    hs = f(inputs["hidden_states"])
    tx = f(inputs["text"])
    in_maps = []
    for c in range(N_CORES):
        m = dict(shared)
        m["xT"] = np.ascontiguousarray(hs[c].T).astype(bf16)
        m["tT"] = np.ascontiguousarray(tx[c].T).astype(bf16)
        in_maps.append(m)
    return in_maps


def kernel(**inputs):
    nc = _build_program()
    in_maps = _host_prep(inputs)
    res = run_bass_kernel_spmd(nc, in_maps, core_ids=list(range(N_CORES)))
    out = np.stack([res.results[c]["out"] for c in range(N_CORES)])
    return out.astype(np.float32)
